# revision 12
# baseline (speedup 1.0000x reference)
"""Hybrid fp8-DoubleRow / bf16 Trainium2 kernel for the additive-attention
glimpse module.

Math (per batch b):
    qp  = query @ Wq.T + bq                       # [E]
    cp  = context @ Wc.T + bc                     # [N, E]
    comb = tanh(qp + cp)                          # [N, E]
    attn = comb @ Wo.T (+ bo, softmax-invariant)  # [N, G]
    w    = softmax(attn, axis=N)                  # [N, G]
    out  = (w.T @ context).reshape(G*Cd)          # [G*Cd]

Shapes: B=256, N=196, Cd=2048, Qd=E=1024, G=8.  Data-parallel over B on 8
cores (32 batches each).

HW notes (probe-measured): an fp8 DoubleRowSwInterleave matmul streams at
1 column/cycle (157 TF/s, 2 k-tiles per pass = exactly 2x bf16), NOT the
0.5 cyc/col the CoreSim cost model claims; per-instruction overhead is
~3.5 ns, and the 256-row SWI stationary load hides under streams of >=
~256 columns.  Hence: the dominant matmul cp.T = Wc @ ctx.T (~26
GFLOP/core) runs with 14 of the 16 contraction k-tiles as fp8e4m3
DR-SWI pairs and 2 k-tiles in bf16 as the accuracy anchor (rel err
~1.93e-2 < 2e-2 gate; pure fp8 would be ~2.07e-2), streamed in
(512, 272)-column chunks (not 2x392) so the SWI weight loads hide.

Wc is pre-scaled by 32 (fp8 subnormal avoidance) and the tanh applies
the 1/32 compensation via its scale parameter.  attn matmuls for all 4
batches of a slab pack into the 4 PE column groups of one PSUM tile;
softmax reads PSUM directly and the output scaling is one full-partition
DVE op per 512-column block.  Slab tails (softmax-weight transposes +
glimpse + output DMA) are software-pipelined one slab behind cp/attn,
and the qp matmuls are interleaved into slab 0's e-loop so the PE is
never DMA-starved at startup.
"""

import numpy as np
import ml_dtypes

BF16 = ml_dtypes.bfloat16
FP8 = ml_dtypes.float8_e4m3  # IEEE-style e4m3, max 240 == TRN FP8_EXP4

B_FULL = 256
N_CTX = 196
CD = 2048
QD = 1024
E = 1024
G = 8
N_CORES = 8
B_LOC = B_FULL // N_CORES  # 32

SLAB_B = 4                  # batches per slab
R_SLAB = SLAB_B * N_CTX     # 784
CHUNKS = ((0, 512), (512, 272))  # psum column chunks within a slab

WC_SCALE = 32.0             # host pre-scale on Wc (both halves)

NE = E // 128    # 8 e-tiles
NCC = CD // 128  # 16 c-tiles
NQ = QD // 128   # 8 q-tiles

FP8_PAIRS = 7    # fp8 DR-SWI pairs (14 k-tiles); the rest are bf16


def build_nc(b_loc=B_LOC, fp8_pairs=FP8_PAIRS, reps=1, probe=None):
    import concourse.mybir as mybir
    import concourse.tile as tile
    from concourse import bacc
    from concourse.masks import make_identity

    f32 = mybir.dt.float32
    bf16 = mybir.dt.bfloat16
    fp8 = mybir.dt.float8e4
    Act = mybir.ActivationFunctionType
    Alu = mybir.AluOpType
    DR = mybir.MatmulPerfMode.DoubleRowSwInterleave

    n8 = 2 * fp8_pairs       # fp8 k-tiles
    n16 = NCC - n8           # bf16 k-tiles
    assert 0 <= n8 <= NCC

    assert b_loc % SLAB_B == 0
    n_slab = b_loc // SLAB_B
    R = b_loc * N_CTX

    nc = bacc.Bacc("TRN2", target_bir_lowering=False, debug=False,
                   num_devices=N_CORES)

    ctx_nat = nc.dram_tensor("ctxn", [R, CD], bf16, kind="ExternalInput").ap()
    ctx8T = nc.dram_tensor("ctx8T", [n8 * 128, R], fp8,
                           kind="ExternalInput").ap()
    wc8T = nc.dram_tensor("wc8T", [fp8_pairs * 128, NE * 256], fp8,
                          kind="ExternalInput").ap()
    if n16:
        ctx16T = nc.dram_tensor("ctx16T", [n16 * 128, R], bf16,
                                kind="ExternalInput").ap()
        wc16T = nc.dram_tensor("wc16T", [n16 * 128, E], bf16,
                               kind="ExternalInput").ap()
    qT = nc.dram_tensor("qT", [QD, b_loc], bf16, kind="ExternalInput").ap()
    WqT = nc.dram_tensor("WqT", [QD, E], bf16, kind="ExternalInput").ap()
    WoT = nc.dram_tensor("WoT", [E, G], bf16, kind="ExternalInput").ap()
    bqc = nc.dram_tensor("bqc", [128, E // 128], f32, kind="ExternalInput").ap()
    out = nc.dram_tensor("out", [b_loc, G * CD], f32, kind="ExternalOutput").ap()

    # batch segments within each psum chunk: (chunk_idx, col0, cols, batch)
    segs = []
    for ci, (co, cw) in enumerate(CHUNKS):
        for j in range(SLAB_B):
            b0, b1 = j * N_CTX, (j + 1) * N_CTX
            lo, hi = max(b0, co), min(b1, co + cw)
            if lo < hi:
                segs.append((ci, lo - co, hi - lo, j, lo - j * N_CTX))

    with tile.TileContext(nc) as tc:
        with (
            tc.tile_pool(name="const", bufs=1) as const_pool,
            tc.tile_pool(name="xt", bufs=2) as xt_pool,
            tc.tile_pool(name="nat", bufs=2) as nat_pool,
            tc.tile_pool(name="comb", bufs=2) as comb_pool,
            tc.tile_pool(name="sm", bufs=8) as sm_pool,
            tc.tile_pool(name="outb", bufs=2) as outb_pool,
            tc.tile_pool(name="pcp", bufs=4, space="PSUM") as pc_pool,
            tc.tile_pool(name="pat", bufs=1, space="PSUM") as pa_pool,
            tc.tile_pool(name="pgl", bufs=2, space="PSUM") as pg_pool,
            tc.tile_pool(name="pta", bufs=1, space="PSUM") as pt_pool,
        ):
            # ---- persistent constants ----
            wo_sb = const_pool.tile([128, NE, G], bf16)
            nc.sync.dma_start(wo_sb[:], WoT.rearrange("(k p) g -> p k g", p=128))
            bqc_sb = const_pool.tile([128, NE], f32)
            nc.sync.dma_start(bqc_sb[:], bqc[:])
            ident = const_pool.tile([128, 128], bf16)
            make_identity(nc, ident[:])

            wc8_sb = const_pool.tile([128, fp8_pairs, NE * 256], fp8)
            wc8r = wc8T.rearrange("(k p) x -> p k x", p=128)
            if n16:
                wc16_sb = const_pool.tile([128, n16, E], bf16)
                wc16r = wc16T.rearrange("(k p) e -> p k e", p=128)

            qpb_sb = const_pool.tile([128, NE, b_loc], f32, tag="qpb")

            # softmax weights indexed by GLOBAL slab row r = 196j + n (so a
            # transpose of a 128-col window lands each batch's weights at the
            # right partitions for the block-diagonal glimpse).  Two
            # persistent ping-pong buffers, zeroed once: the per-slab writes
            # hit the same cells every slab, everything else must stay 0.
            wex_g = [const_pool.tile([128, R_SLAB], bf16, name=f"wexg{i}")
                     for i in range(2)]
            for t in wex_g:
                nc.gpsimd.memset(t[:], 0.0)
            wcat = [const_pool.tile([128, 7, 128], bf16, name=f"wcat{i}")
                    for i in range(2)]
            # glimpse windows: rows r0+128w .. +wlen (w6 is the 16-row tail)
            WINS = [(w, 128 * w, 128 if w < 6 else R_SLAB - 768)
                    for w in range(7)]

            def load_cp_weights():
                # per-pair DMAs so slab-0 cp only waits on pair 0
                for i in range(fp8_pairs):
                    nc.sync.dma_start(wc8_sb[:, i:i + 1, :], wc8r[:, i:i + 1, :])
                if n16:
                    nc.sync.dma_start(wc16_sb[:], wc16r[:])

            def one_pass(first=False, out_scale=1.0):
                # qp-phase DMAs (issued before the big cp-weight loads)
                qt_sb = wq_pool.tile([128, NQ, b_loc], bf16, tag="qt")
                nc.sync.dma_start(qt_sb[:], qT.rearrange("(k p) b -> p k b", p=128))
                wq_sbs = []
                for e in range(NE):
                    wq_sb = wq_pool.tile([128, NQ, 128], bf16, tag="wqe")
                    nc.sync.dma_start(
                        wq_sb[:],
                        WqT[:, e * 128:(e + 1) * 128].rearrange(
                            "(k p) m -> p k m", p=128))
                    wq_sbs.append(wq_sb)
                if first:
                    load_cp_weights()

                # ---- main loop over 4-batch slabs ----
                for s in range(n_slab):
                    r0 = s * R_SLAB
                    xt8 = xt_pool.tile([128, n8, R_SLAB], fp8, tag="xt8")
                    c8r = ctx8T.rearrange("(k p) r -> p k r", p=128)
                    for k in range(0, n8, 2):
                        nc.sync.dma_start(
                            xt8[:, k:k + 2, :],
                            c8r[:, k:k + 2, r0:r0 + R_SLAB])
                    if n16:
                        xt16 = xt_pool.tile([128, n16, R_SLAB], bf16, tag="xt16")
                        c16r = ctx16T.rearrange("(k p) r -> p k r", p=128)
                        for k in range(n16):
                            nc.sync.dma_start(
                                xt16[:, k, :],
                                c16r[:, k, r0:r0 + R_SLAB])

                    if probe is None:
                        nat = nat_pool.tile([128, 7, CD], bf16, tag="nat")
                        for w, wr0, wlen in WINS:
                            nc.sync.dma_start(
                                nat[0:wlen, w, :],
                                ctx_nat[r0 + wr0:r0 + wr0 + wlen, :])

                    # ---- cp + tanh -> comb, (512, 272)-col psum chunks ----
                    comb = comb_pool.tile([128, NE, R_SLAB], bf16, tag="comb")
                    for e in range(NE):
                        if s == 0:
                            # qp e-tile interleaved here so slab-0 cp isn't
                            # blocked behind the whole qp phase at startup
                            pq = pa_pool.tile([128, b_loc], f32, tag="pat")
                            for k in range(NQ):
                                nc.tensor.matmul(
                                    pq[:], wq_sbs[e][:, k, :], qt_sb[:, k, :],
                                    start=(k == 0), stop=(k == NQ - 1))
                            nc.vector.tensor_scalar_add(
                                qpb_sb[:, e, :], pq[:], bqc_sb[:, e:e + 1])
                        es = slice(e * 128, (e + 1) * 128)
                        pcs = [pc_pool.tile([128, cw], f32, tag="pcp",
                                            name=f"pc{ci}")
                               for ci, (co, cw) in enumerate(CHUNKS)]
                        n_mm = fp8_pairs + n16
                        mm_i = 0
                        for i in range(fp8_pairs):
                            ks = slice(2 * i, 2 * i + 2)
                            w_ap = wc8_sb[:, i, e * 256:(e + 1) * 256]
                            for h, (co, cw) in enumerate(CHUNKS):
                                nc.tensor.matmul(
                                    pcs[h][:], w_ap,
                                    xt8[:, ks, co:co + cw],
                                    start=(mm_i == 0), stop=(mm_i == n_mm - 1),
                                    perf_mode=DR,
                                )
                            mm_i += 1
                        for j in range(n16):
                            for h, (co, cw) in enumerate(CHUNKS):
                                nc.tensor.matmul(
                                    pcs[h][:], wc16_sb[:, j, es],
                                    xt16[:, j, co:co + cw],
                                    start=(mm_i == 0), stop=(mm_i == n_mm - 1),
                                )
                            mm_i += 1
                        if probe == "mm_cycle":
                            continue
                        for ci, lo, ln, j, nb in segs:
                            nc.scalar.activation(
                                comb[:, e, j * N_CTX + nb:j * N_CTX + nb + ln],
                                pcs[ci][:, lo:lo + ln],
                                Act.Tanh,
                                bias=qpb_sb[:, e, s * SLAB_B + j:
                                            s * SLAB_B + j + 1],
                                scale=1.0 / WC_SCALE,
                            )

                    if probe is not None:
                        continue

                    # ---- attn.T = WoT.T @ comb.T: all 4 batches packed into
                    # the 4 PE column groups of ONE [128, 196] psum tile ----
                    pa = pa_pool.tile([128, N_CTX], f32, tag="pat")
                    for e in range(NE):
                        for j4 in range(SLAB_B):
                            nc.tensor.matmul(
                                pa[32 * j4:32 * j4 + G, :],
                                wo_sb[:, e, :],
                                comb[:, e, j4 * N_CTX:(j4 + 1) * N_CTX],
                                start=(e == 0), stop=(e == NE - 1),
                                tile_position=(0, 32 * j4),
                                skip_group_check=True,
                            )

                    # ---- softmax for all 4 batches, stacked at partition
                    # bases 0/32/64/96 (32-aligned), straight from PSUM;
                    # exp output goes to the global-r-indexed weight tile ----
                    wg = wex_g[s % 2]
                    ssum = sm_pool.tile([128, 1], f32, tag="ssum")
                    for j in range(SLAB_B):
                        seg = pa[32 * j:32 * j + G, :]
                        nmx = sm_pool.tile([G, 1], f32, tag="nmx")
                        nc.vector.tensor_reduce(
                            nmx[:], seg, axis=mybir.AxisListType.X,
                            op=Alu.max, negate=True)
                        nc.scalar.activation(
                            wg[32 * j:32 * j + G,
                               j * N_CTX:(j + 1) * N_CTX],
                            seg, Act.Exp,
                            bias=nmx[:], accum_out=ssum[32 * j:32 * j + G, :])
                    rs = sm_pool.tile([128, 1], f32, tag="rs")
                    nc.vector.reciprocal(rs[:], ssum[:])
                    if out_scale != 1.0:
                        nc.vector.tensor_scalar_mul(
                            rs[:], rs[:], float(out_scale))
                    # tail (transposes+glimpse+out) for the PREVIOUS slab is
                    # emitted after this slab's cp/attn so the PE never waits
                    # on the softmax ACT/DVE chain.
                    if pending[0] is not None:
                        emit_tail(*pending[0])
                    pending[0] = (s, wg, rs, nat)

                if probe is None and pending[0] is not None:
                    emit_tail(*pending[0])
                    pending[0] = None

            def emit_tail(s, wg, rs_full, nat):
                # per-window transposes of the global-r-indexed weights: the
                # output of window w is the dense [rows, 128] block-diagonal
                # stationary (batch j's weights in cols 32j..32j+G, zeros
                # elsewhere from the one-time memset)
                wc = wcat[s % 2]
                # all 7 window transposes land in ONE psum bank, then a
                # single DVE copy moves the whole block-diagonal stationary
                # to SBUF (avoids PE<->DVE ping-pong through a shallow ring)
                pt_all = pt_pool.tile([128, 7, 128], bf16, tag="pta")
                for w, wr0, wlen in WINS:
                    nc.tensor.transpose(pt_all[0:wlen, w, :],
                                        wg[:, wr0:wr0 + wlen], ident[:])
                nc.vector.tensor_copy(wc[:], pt_all[:])

                # ---- glimpse: all 4 batches per pass via the block-diagonal
                # stationary; 7 windows instead of 8 half-empty passes ----
                outb = outb_pool.tile([128, CD], f32, tag="outb")
                for cc in range(CD // 512):
                    pg = pg_pool.tile([128, 512], f32, tag="pgl")
                    for w, wr0, wlen in WINS:
                        nc.tensor.matmul(
                            pg[:], wc[0:wlen, w, :],
                            nat[0:wlen, w, cc * 512:(cc + 1) * 512],
                            start=(w == 0), stop=(w == 6))
                    # one full-partition scale: garbage rows (outside the
                    # 32j..32j+G groups) are scaled by garbage but never read
                    nc.vector.tensor_scalar_mul(
                        outb[:, cc * 512:(cc + 1) * 512], pg[:], rs_full)

                for j in range(SLAB_B):
                    nc.gpsimd.dma_start(
                        out[s * SLAB_B + j, :].rearrange(
                            "(g c) -> g c", g=G),
                        outb[32 * j:32 * j + G, :])

            with tc.tile_pool(name="wq", bufs=NE) as wq_pool:
                pending = [None]
                for _rep in range(reps):
                    one_pass(first=(_rep == 0))

    nc.compile()
    return nc


_NC_CACHE = {}


def _get_nc(b_loc=B_LOC, fp8_pairs=FP8_PAIRS):
    key = (b_loc, fp8_pairs)
    if key not in _NC_CACHE:
        _NC_CACHE[key] = build_nc(b_loc, fp8_pairs=fp8_pairs)
    return _NC_CACHE[key]


def _swi_interleave(WcTs):
    """[2*P*128, E] weight rows -> SwInterleave fp8 weight layout
    [P*128, NE*256]: per pair tile, free dim holds e-tiles of
    (A[:, ::-1], B[:, ::-1]) column-interleaved."""
    P = WcTs.shape[0] // 256
    w8 = WcTs.astype(FP8)
    out = np.empty((P, 128, NE, 256), dtype=FP8)
    for i in range(P):
        A = np.asarray(w8[(2 * i) * 128:(2 * i + 1) * 128]).reshape(128, NE, 128)
        B = np.asarray(
            w8[(2 * i + 1) * 128:(2 * i + 2) * 128]).reshape(128, NE, 128)
        out[i, :, :, 0::2] = A[:, :, ::-1]
        out[i, :, :, 1::2] = B[:, :, ::-1]
    return np.ascontiguousarray(out.reshape(P * 128, NE * 256))


def make_in_maps(context, query, Wq, bq, Wc, bc, Wo, bo, b_loc=B_LOC,
                 n_cores=N_CORES, fp8_pairs=FP8_PAIRS):
    """Host-side prep: dtype conversion, transposes, quantization, sharding."""
    n8 = 2 * fp8_pairs
    n16 = NCC - n8
    c_split = n8 * 128

    context = np.asarray(context, dtype=np.float32)
    query = np.asarray(query)
    Wq = np.asarray(Wq)
    bq, bc_ = np.asarray(bq), np.asarray(bc)
    Wc = np.asarray(Wc, dtype=np.float32)
    Wo = np.asarray(Wo)

    ctx_bf = np.ascontiguousarray(context).astype(BF16)
    WqT = np.ascontiguousarray(Wq.T).astype(BF16)
    WoT = np.ascontiguousarray(np.asarray(Wo).T).astype(BF16)
    bqc = np.ascontiguousarray(
        (bq + bc_).astype(np.float32).reshape(E // 128, 128).T)

    WcTs = np.ascontiguousarray(Wc.T * WC_SCALE)  # [Cd, E], pre-scaled
    wc8T = _swi_interleave(WcTs[:c_split])
    wc16T = WcTs[c_split:].astype(BF16) if n16 else None

    in_maps = []
    for i in range(n_cores):
        b0 = i * b_loc
        ctx_i = context[b0:b0 + b_loc].reshape(b_loc * N_CTX, CD)
        m = dict(
            ctxn=ctx_bf[b0:b0 + b_loc].reshape(b_loc * N_CTX, CD),
            ctx8T=np.ascontiguousarray(ctx_i[:, :c_split].T).astype(FP8),
            wc8T=wc8T,
            qT=np.ascontiguousarray(query[b0:b0 + b_loc].T).astype(BF16),
            WqT=WqT, WoT=WoT, bqc=bqc,
        )
        if n16:
            m["ctx16T"] = np.ascontiguousarray(
                ctx_i[:, c_split:].T).astype(BF16)
            m["wc16T"] = wc16T
        in_maps.append(m)
    return in_maps


def kernel(context, query, Wq, bq, Wc, bc, Wo, bo, fp8_pairs=FP8_PAIRS):
    from concourse.bass_utils import run_bass_kernel_spmd

    assert context.shape == (B_FULL, N_CTX, CD)
    nc = _get_nc(fp8_pairs=fp8_pairs)
    in_maps = make_in_maps(context, query, Wq, bq, Wc, bc, Wo, bo,
                           fp8_pairs=fp8_pairs)
    res = run_bass_kernel_spmd(nc, in_maps, core_ids=list(range(N_CORES)))
    return np.concatenate([res.results[i]["out"] for i in range(N_CORES)],
                          axis=0)


# revision 13
# speedup vs baseline: 1.1533x; 1.1533x over previous
"""Hybrid fp8-DoubleRow / bf16 Trainium2 kernel for the additive-attention
glimpse module.

Math (per batch b):
    qp  = query @ Wq.T + bq                       # [E]
    cp  = context @ Wc.T + bc                     # [N, E]
    comb = tanh(qp + cp)                          # [N, E]
    attn = comb @ Wo.T (+ bo, softmax-invariant)  # [N, G]
    w    = softmax(attn, axis=N)                  # [N, G]
    out  = (w.T @ context).reshape(G*Cd)          # [G*Cd]

Shapes: B=256, N=196, Cd=2048, Qd=E=1024, G=8.  Data-parallel over B on 8
cores (32 batches each).

HW notes (probe-measured): an fp8 DoubleRowSwInterleave matmul streams at
1 column/cycle (157 TF/s, 2 k-tiles per pass = exactly 2x bf16), NOT the
0.5 cyc/col the CoreSim cost model claims; per-instruction overhead is
~3.5 ns, and the 256-row SWI stationary load hides under streams of >=
~256 columns.  Hence: the dominant matmul cp.T = Wc @ ctx.T (~26
GFLOP/core) runs with 14 of the 16 contraction k-tiles as fp8e4m3
DR-SWI pairs and 2 k-tiles in bf16 as the accuracy anchor (rel err
~1.93e-2 < 2e-2 gate; pure fp8 would be ~2.07e-2), streamed in
(512, 272)-column chunks (not 2x392) so the SWI weight loads hide.

Wc is pre-scaled by 32 (fp8 subnormal avoidance) and the tanh applies
the 1/32 compensation via its scale parameter.  attn matmuls for all 4
batches of a slab pack into the 4 PE column groups of one PSUM tile;
softmax reads PSUM directly and the output scaling is one full-partition
DVE op per 512-column block.  Slab tails (softmax-weight transposes +
glimpse + output DMA) are software-pipelined one slab behind cp/attn,
and the qp matmuls are interleaved into slab 0's e-loop so the PE is
never DMA-starved at startup.
"""

import numpy as np
import ml_dtypes

BF16 = ml_dtypes.bfloat16
FP8 = ml_dtypes.float8_e4m3  # IEEE-style e4m3, max 240 == TRN FP8_EXP4

B_FULL = 256
N_CTX = 196
CD = 2048
QD = 1024
E = 1024
G = 8
N_CORES = 8
B_LOC = B_FULL // N_CORES  # 32

SLAB_B = 4                  # batches per slab
R_SLAB = SLAB_B * N_CTX     # 784
CHUNKS = ((0, 512), (512, 272))  # psum column chunks within a slab

WC_SCALE = 32.0             # host pre-scale on Wc (both halves)

NE = E // 128    # 8 e-tiles
NCC = CD // 128  # 16 c-tiles
NQ = QD // 128   # 8 q-tiles

FP8_PAIRS = 7    # fp8 DR-SWI pairs (14 k-tiles); the rest are bf16


def build_nc(b_loc=B_LOC, fp8_pairs=FP8_PAIRS, reps=1, probe=None):
    import concourse.mybir as mybir
    import concourse.tile as tile
    from concourse import bacc
    from concourse.masks import make_identity

    f32 = mybir.dt.float32
    bf16 = mybir.dt.bfloat16
    fp8 = mybir.dt.float8e4
    Act = mybir.ActivationFunctionType
    Alu = mybir.AluOpType
    DR = mybir.MatmulPerfMode.DoubleRowSwInterleave

    n8 = 2 * fp8_pairs       # fp8 k-tiles
    n16 = NCC - n8           # bf16 k-tiles
    assert 0 <= n8 <= NCC

    assert b_loc % SLAB_B == 0
    n_slab = b_loc // SLAB_B
    R = b_loc * N_CTX

    nc = bacc.Bacc("TRN2", target_bir_lowering=False, debug=False,
                   num_devices=N_CORES)

    ctx_nat = nc.dram_tensor("ctxn", [R, CD], bf16, kind="ExternalInput").ap()
    ctx8T = nc.dram_tensor("ctx8T", [n8 * 128, R], fp8,
                           kind="ExternalInput").ap()
    wc8T = nc.dram_tensor("wc8T", [fp8_pairs * 128, NE * 256], fp8,
                          kind="ExternalInput").ap()
    if n16:
        ctx16T = nc.dram_tensor("ctx16T", [n16 * 128, R], bf16,
                                kind="ExternalInput").ap()
        wc16T = nc.dram_tensor("wc16T", [n16 * 128, E], bf16,
                               kind="ExternalInput").ap()
    qT = nc.dram_tensor("qT", [QD, b_loc], bf16, kind="ExternalInput").ap()
    WqT = nc.dram_tensor("WqT", [QD, E], bf16, kind="ExternalInput").ap()
    WoT = nc.dram_tensor("WoT", [E, G], bf16, kind="ExternalInput").ap()
    bqc = nc.dram_tensor("bqc", [128, E // 128], f32, kind="ExternalInput").ap()
    out = nc.dram_tensor("out", [b_loc, G * CD], f32, kind="ExternalOutput").ap()

    # batch segments within each psum chunk: (chunk_idx, col0, cols, batch)
    segs = []
    for ci, (co, cw) in enumerate(CHUNKS):
        for j in range(SLAB_B):
            b0, b1 = j * N_CTX, (j + 1) * N_CTX
            lo, hi = max(b0, co), min(b1, co + cw)
            if lo < hi:
                segs.append((ci, lo - co, hi - lo, j, lo - j * N_CTX))

    with tile.TileContext(nc) as tc:
        with (
            tc.tile_pool(name="const", bufs=1) as const_pool,
            tc.tile_pool(name="xt", bufs=2) as xt_pool,
            tc.tile_pool(name="nat", bufs=2) as nat_pool,
            tc.tile_pool(name="comb", bufs=2) as comb_pool,
            tc.tile_pool(name="sm", bufs=8) as sm_pool,
            tc.tile_pool(name="wl", bufs=4) as wl_pool,
            tc.tile_pool(name="outb", bufs=2) as outb_pool,
            tc.tile_pool(name="pcp", bufs=4, space="PSUM") as pc_pool,
            tc.tile_pool(name="pat", bufs=2, space="PSUM") as pa_pool,
            tc.tile_pool(name="pgl", bufs=2, space="PSUM") as pg_pool,
        ):
            # ---- persistent constants ----
            wo_sb = const_pool.tile([128, NE, G], bf16)
            nc.sync.dma_start(wo_sb[:], WoT.rearrange("(k p) g -> p k g", p=128))
            bqc_sb = const_pool.tile([128, NE], f32)
            nc.sync.dma_start(bqc_sb[:], bqc[:])
            ident = const_pool.tile([128, 128], bf16)
            make_identity(nc, ident[:])

            wc8_sb = const_pool.tile([128, fp8_pairs, NE * 256], fp8)
            wc8r = wc8T.rearrange("(k p) x -> p k x", p=128)
            if n16:
                wc16_sb = const_pool.tile([128, n16, E], bf16)
                wc16r = wc16T.rearrange("(k p) e -> p k e", p=128)

            qpb_sb = const_pool.tile([128, NE, b_loc], f32, tag="qpb")

            def load_cp_weights():
                # per-pair DMAs so slab-0 cp only waits on pair 0
                for i in range(fp8_pairs):
                    nc.sync.dma_start(wc8_sb[:, i:i + 1, :], wc8r[:, i:i + 1, :])
                if n16:
                    nc.sync.dma_start(wc16_sb[:], wc16r[:])

            def one_pass(first=False, out_scale=1.0):
                # qp-phase DMAs (issued before the big cp-weight loads)
                qt_sb = wq_pool.tile([128, NQ, b_loc], bf16, tag="qt")
                nc.sync.dma_start(qt_sb[:], qT.rearrange("(k p) b -> p k b", p=128))
                wq_sbs = []
                for e in range(NE):
                    wq_sb = wq_pool.tile([128, NQ, 128], bf16, tag="wqe")
                    nc.sync.dma_start(
                        wq_sb[:],
                        WqT[:, e * 128:(e + 1) * 128].rearrange(
                            "(k p) m -> p k m", p=128))
                    wq_sbs.append(wq_sb)
                if first:
                    load_cp_weights()

                # ---- main loop over 4-batch slabs ----
                for s in range(n_slab):
                    r0 = s * R_SLAB
                    xt8 = xt_pool.tile([128, n8, R_SLAB], fp8, tag="xt8")
                    c8r = ctx8T.rearrange("(k p) r -> p k r", p=128)
                    for k in range(0, n8, 2):
                        nc.sync.dma_start(
                            xt8[:, k:k + 2, :],
                            c8r[:, k:k + 2, r0:r0 + R_SLAB])
                    if n16:
                        xt16 = xt_pool.tile([128, n16, R_SLAB], bf16, tag="xt16")
                        c16r = ctx16T.rearrange("(k p) r -> p k r", p=128)
                        for k in range(n16):
                            nc.sync.dma_start(
                                xt16[:, k, :],
                                c16r[:, k, r0:r0 + R_SLAB])

                    if probe is None:
                        nat_a = nat_pool.tile([128, SLAB_B, CD], bf16, tag="nat_a")
                        nat_b = nat_pool.tile([68, SLAB_B, CD], bf16, tag="nat_b")
                        for j in range(SLAB_B):
                            rb = r0 + j * N_CTX
                            nc.sync.dma_start(nat_a[:, j, :],
                                              ctx_nat[rb:rb + 128, :])
                            nc.sync.dma_start(nat_b[:, j, :],
                                              ctx_nat[rb + 128:rb + N_CTX, :])

                    # ---- cp + tanh -> comb, (512, 272)-col psum chunks ----
                    comb = comb_pool.tile([128, NE, R_SLAB], bf16, tag="comb")
                    for e in range(NE):
                        if s == 0:
                            # qp e-tile interleaved here so slab-0 cp isn't
                            # blocked behind the whole qp phase at startup
                            pq = pa_pool.tile([128, b_loc], f32, tag="pat")
                            for k in range(NQ):
                                nc.tensor.matmul(
                                    pq[:], wq_sbs[e][:, k, :], qt_sb[:, k, :],
                                    start=(k == 0), stop=(k == NQ - 1))
                            nc.vector.tensor_scalar_add(
                                qpb_sb[:, e, :], pq[:], bqc_sb[:, e:e + 1])
                        es = slice(e * 128, (e + 1) * 128)
                        pcs = [pc_pool.tile([128, cw], f32, tag="pcp",
                                            name=f"pc{ci}")
                               for ci, (co, cw) in enumerate(CHUNKS)]
                        n_mm = fp8_pairs + n16
                        mm_i = 0
                        for i in range(fp8_pairs):
                            ks = slice(2 * i, 2 * i + 2)
                            w_ap = wc8_sb[:, i, e * 256:(e + 1) * 256]
                            for h, (co, cw) in enumerate(CHUNKS):
                                nc.tensor.matmul(
                                    pcs[h][:], w_ap,
                                    xt8[:, ks, co:co + cw],
                                    start=(mm_i == 0), stop=(mm_i == n_mm - 1),
                                    perf_mode=DR,
                                )
                            mm_i += 1
                        for j in range(n16):
                            for h, (co, cw) in enumerate(CHUNKS):
                                nc.tensor.matmul(
                                    pcs[h][:], wc16_sb[:, j, es],
                                    xt16[:, j, co:co + cw],
                                    start=(mm_i == 0), stop=(mm_i == n_mm - 1),
                                )
                            mm_i += 1
                        if probe == "mm_cycle":
                            continue
                        for ci, lo, ln, j, nb in segs:
                            nc.scalar.activation(
                                comb[:, e, j * N_CTX + nb:j * N_CTX + nb + ln],
                                pcs[ci][:, lo:lo + ln],
                                Act.Tanh,
                                bias=qpb_sb[:, e, s * SLAB_B + j:
                                            s * SLAB_B + j + 1],
                                scale=1.0 / WC_SCALE,
                            )

                    if probe is not None:
                        continue

                    # ---- attn.T = WoT.T @ comb.T: all 4 batches packed into
                    # the 4 PE column groups of ONE [128, 196] psum tile ----
                    pa = pa_pool.tile([128, N_CTX], f32, tag="pat")
                    for e in range(NE):
                        for j4 in range(SLAB_B):
                            nc.tensor.matmul(
                                pa[32 * j4:32 * j4 + G, :],
                                wo_sb[:, e, :],
                                comb[:, e, j4 * N_CTX:(j4 + 1) * N_CTX],
                                start=(e == 0), stop=(e == NE - 1),
                                tile_position=(0, 32 * j4),
                                skip_group_check=True,
                            )

                    # ---- softmax for all 4 batches, stacked at partition
                    # bases 0/32/64/96 (32-aligned), straight from PSUM ----
                    wexs = sm_pool.tile([128, N_CTX], bf16, tag="wexs")
                    ssum = sm_pool.tile([128, 1], f32, tag="ssum")
                    for j in range(SLAB_B):
                        seg = pa[32 * j:32 * j + G, :]
                        nmx = sm_pool.tile([G, 1], f32, tag="nmx")
                        nc.vector.tensor_reduce(
                            nmx[:], seg, axis=mybir.AxisListType.X,
                            op=Alu.max, negate=True)
                        nc.scalar.activation(
                            wexs[32 * j:32 * j + G, :], seg, Act.Exp,
                            bias=nmx[:], accum_out=ssum[32 * j:32 * j + G, :])
                    rs = sm_pool.tile([128, 1], f32, tag="rs")
                    nc.vector.reciprocal(rs[:], ssum[:])
                    if out_scale != 1.0:
                        nc.vector.tensor_scalar_mul(
                            rs[:], rs[:], float(out_scale))
                    # tail (transposes+glimpse+out) for the PREVIOUS slab is
                    # emitted after this slab's cp/attn so the PE never waits
                    # on the softmax ACT/DVE chain.
                    if pending[0] is not None:
                        emit_tail(*pending[0])
                    pending[0] = (s, wexs, rs, nat_a, nat_b)

                if probe is None and pending[0] is not None:
                    emit_tail(*pending[0])
                    pending[0] = None

            def emit_tail(s, wexs, rs_full, nat_a, nat_b):
                # one stacked transpose pair for the whole slab
                wla = wl_pool.tile([128, 128], bf16, tag="wla")
                wlb = wl_pool.tile([68, 128], bf16, tag="wlb")
                pta = pg_pool.tile([128, 128], bf16, tag="pgl")
                nc.tensor.transpose(pta[:], wexs[:, 0:128], ident[:])
                nc.vector.tensor_copy(wla[:], pta[:])
                ptb = pg_pool.tile([68, 128], bf16, tag="pgl")
                nc.tensor.transpose(ptb[:], wexs[:, 128:N_CTX], ident[:])
                nc.vector.tensor_copy(wlb[:], ptb[:])
                wls = [(wla[:, 32 * j:32 * j + G],
                        wlb[:, 32 * j:32 * j + G])
                       for j in range(SLAB_B)]

                # ---- glimpse: 4 batches via PE column tiling ----
                outb = outb_pool.tile([128, CD], f32, tag="outb")
                for cc in range(CD // 512):
                    pg = pg_pool.tile([128, 512], f32, tag="pgl")
                    for j in range(SLAB_B):
                        nc.tensor.matmul(
                            pg[32 * j:32 * j + G, :], wls[j][0],
                            nat_a[:, j, cc * 512:(cc + 1) * 512],
                            start=True, stop=False,
                            tile_position=(0, 32 * j),
                            skip_group_check=True)
                    for j in range(SLAB_B):
                        nc.tensor.matmul(
                            pg[32 * j:32 * j + G, :], wls[j][1],
                            nat_b[:, j, cc * 512:(cc + 1) * 512],
                            start=False, stop=True,
                            tile_position=(0, 32 * j),
                            skip_group_check=True)
                    # one full-partition scale: garbage rows (outside the
                    # 32j..32j+G groups) are scaled by garbage but never read
                    nc.vector.tensor_scalar_mul(
                        outb[:, cc * 512:(cc + 1) * 512], pg[:], rs_full)

                for j in range(SLAB_B):
                    nc.gpsimd.dma_start(
                        out[s * SLAB_B + j, :].rearrange(
                            "(g c) -> g c", g=G),
                        outb[32 * j:32 * j + G, :])

            with tc.tile_pool(name="wq", bufs=NE) as wq_pool:
                pending = [None]
                for _rep in range(reps):
                    one_pass(first=(_rep == 0))

    nc.compile()
    return nc


_NC_CACHE = {}


def _get_nc(b_loc=B_LOC, fp8_pairs=FP8_PAIRS):
    key = (b_loc, fp8_pairs)
    if key not in _NC_CACHE:
        _NC_CACHE[key] = build_nc(b_loc, fp8_pairs=fp8_pairs)
    return _NC_CACHE[key]


def _swi_interleave(WcTs):
    """[2*P*128, E] weight rows -> SwInterleave fp8 weight layout
    [P*128, NE*256]: per pair tile, free dim holds e-tiles of
    (A[:, ::-1], B[:, ::-1]) column-interleaved."""
    P = WcTs.shape[0] // 256
    w8 = WcTs.astype(FP8)
    out = np.empty((P, 128, NE, 256), dtype=FP8)
    for i in range(P):
        A = np.asarray(w8[(2 * i) * 128:(2 * i + 1) * 128]).reshape(128, NE, 128)
        B = np.asarray(
            w8[(2 * i + 1) * 128:(2 * i + 2) * 128]).reshape(128, NE, 128)
        out[i, :, :, 0::2] = A[:, :, ::-1]
        out[i, :, :, 1::2] = B[:, :, ::-1]
    return np.ascontiguousarray(out.reshape(P * 128, NE * 256))


def make_in_maps(context, query, Wq, bq, Wc, bc, Wo, bo, b_loc=B_LOC,
                 n_cores=N_CORES, fp8_pairs=FP8_PAIRS):
    """Host-side prep: dtype conversion, transposes, quantization, sharding."""
    n8 = 2 * fp8_pairs
    n16 = NCC - n8
    c_split = n8 * 128

    context = np.asarray(context, dtype=np.float32)
    query = np.asarray(query)
    Wq = np.asarray(Wq)
    bq, bc_ = np.asarray(bq), np.asarray(bc)
    Wc = np.asarray(Wc, dtype=np.float32)
    Wo = np.asarray(Wo)

    ctx_bf = np.ascontiguousarray(context).astype(BF16)
    WqT = np.ascontiguousarray(Wq.T).astype(BF16)
    WoT = np.ascontiguousarray(np.asarray(Wo).T).astype(BF16)
    bqc = np.ascontiguousarray(
        (bq + bc_).astype(np.float32).reshape(E // 128, 128).T)

    WcTs = np.ascontiguousarray(Wc.T * WC_SCALE)  # [Cd, E], pre-scaled
    wc8T = _swi_interleave(WcTs[:c_split])
    wc16T = WcTs[c_split:].astype(BF16) if n16 else None

    in_maps = []
    for i in range(n_cores):
        b0 = i * b_loc
        ctx_i = context[b0:b0 + b_loc].reshape(b_loc * N_CTX, CD)
        m = dict(
            ctxn=ctx_bf[b0:b0 + b_loc].reshape(b_loc * N_CTX, CD),
            ctx8T=np.ascontiguousarray(ctx_i[:, :c_split].T).astype(FP8),
            wc8T=wc8T,
            qT=np.ascontiguousarray(query[b0:b0 + b_loc].T).astype(BF16),
            WqT=WqT, WoT=WoT, bqc=bqc,
        )
        if n16:
            m["ctx16T"] = np.ascontiguousarray(
                ctx_i[:, c_split:].T).astype(BF16)
            m["wc16T"] = wc16T
        in_maps.append(m)
    return in_maps


def kernel(context, query, Wq, bq, Wc, bc, Wo, bo, fp8_pairs=FP8_PAIRS):
    from concourse.bass_utils import run_bass_kernel_spmd

    assert context.shape == (B_FULL, N_CTX, CD)
    nc = _get_nc(fp8_pairs=fp8_pairs)
    in_maps = make_in_maps(context, query, Wq, bq, Wc, bc, Wo, bo,
                           fp8_pairs=fp8_pairs)
    res = run_bass_kernel_spmd(nc, in_maps, core_ids=list(range(N_CORES)))
    return np.concatenate([res.results[i]["out"] for i in range(N_CORES)],
                          axis=0)
